# revision 35
# baseline (speedup 1.0000x reference)
"""GAT message-passing kernel for 8 Trainium2 NeuronCores (Bass/Tile).

Strategy ("route edges by dst ownership"):
  - Host sorts edges by dst, partitions nodes into 8 equal ranges; each core
    owns all edges whose dst falls in its range, so segment-softmax and
    scatter-sum are fully core-local (no collectives).
  - Reassociation: epaths = y1[src] + eft@W2 + y3[dst] + b with y1 = nft@W1.
    Since sum(att)=1 per (node, head), the y3[dst] part of the aggregated
    message is exactly +y3[dst], added once per node in phase 3.  Softmax is
    computed without max-subtraction; a fixed shift exp(a-7) keeps the
    unnormalized weights in fp16 range (softmax is shift-invariant).
  - Phase 1 (per core, all nodes, fp16): G table rows [y1 | qa], R table rows
    [r]; qa = a1 + y1.A2, r = y3.A2.
  - Phase 2, per tile of 128 edges (grouped per owning 128-node dst block,
    per-block tile counts fixed across cores at the max over cores):
    dma_gather G rows by src (2048-idx calls, 4 SWDGE queues); matmuls with
    the transposed edge-feature tile stationary produce eft@W2 (PSUM, 4-tile
    batches) and eft@(W2.A2) logits; r[dst] is added by a one-hot matmul
    whose one-hot is built ON DEVICE (broadcast matmul of dstloc + is_equal
    against a partition iota); logits -> leaky_relu (DVE) -> exp(a-7)
    (ACT, fp16); the scatter one-hot P is also built on device (is_equal of
    a free-dim iota against per-partition dstloc); a scatter matmul
    accumulates [agg | s] for the owning 128-node block.
  - Phase 3 (per node block, fp16 operands): agg/s, transpose via PE,
    += nft@W3 (matmul accumulate) and += nft, relu, store transposed.
"""

import sys
import numpy as np

for _p in ("/opt/trn_rl_repo",):
    if _p not in sys.path:
        sys.path.append(_p)

import concourse.bacc as bacc
import concourse.bass as bass
import concourse.mybir as mybir
from concourse.tile import TileContext
from concourse import bass_utils

F = 128
H = 8
DH = 16
F2 = F + H  # 136
NCORES = 8
EXP_SHIFT = 7.0  # exp(a - shift); softmax-invariant, keeps u in fp16 normal range
PREC = "f16"     # "f16" (fast path) or "f32"
GCHUNK = 8       # tiles per gather call / eft chunk


def build_nc(n_nodes, npc, tiles, has_bias, prec=PREC, debug=False):
    nb = npc // 128                  # node block slots per core
    assert len(tiles) == nb
    ntiles = sum(tiles)              # edge tiles per core
    epad = ntiles * 128              # padded edge count per core
    nch = (n_nodes + 127) // 128     # phase-1 node chunks
    npad = nch * 128 + 128           # +128 rows; sentinel lives at row nch*128
    dt = mybir.dt
    f16 = prec == "f16"
    edt = dt.float16 if f16 else dt.float32
    # G row layout: f16 -> fp16[y1(128) | qa-as-f32-bits(16) | pad] row 512 B
    #               f32 -> fp32[y1(128) | qa(8) | pad] row 768 B
    gdt = dt.float16 if f16 else dt.float32
    grow = 256 if f16 else 192
    nchT = nch + 1                   # chunk slots per partition (+1 sentinel)
    shift = EXP_SHIFT if f16 else 0.0

    # flat tile schedule
    blk_of, jpos, jlast = [], [], []
    for b, tb in enumerate(tiles):
        for j in range(tb):
            blk_of.append(b)
            jpos.append(j)
            jlast.append(j == tb - 1)

    nc = bacc.Bacc("TRN2", target_bir_lowering=False, debug=False,
                   num_devices=NCORES, num_swdge_queues=4)

    # ---- inputs (identical shapes on every core) ----
    nftT_full = nc.dram_tensor("nftT_full", (F, npad), edt, kind="ExternalInput")
    nftT_cc = nc.dram_tensor("nftT_cc", (128, nb, 2, 128), edt, kind="ExternalInput")
    # epp: per gather-group interleave [eft | Pcat | PTcat] as raw bytes
    esz = 2 if f16 else 4
    osz = 1 if f16 else 4
    eppE = GCHUNK * 128 * esz        # eft bytes per partition per group
    eppO = GCHUNK * 128 * osz        # one-hot bytes per partition per group
    eppB = eppE + 2 * eppO
    ngrp = (ntiles + GCHUNK - 1) // GCHUNK
    epp = nc.dram_tensor("epp", (128, ngrp, eppB), dt.uint8, kind="ExternalInput")
    W_path = nc.dram_tensor("W_path", (3 * F, F), dt.float32, kind="ExternalInput")
    W_pathT = nc.dram_tensor("W_pathT", (F, 3 * F), dt.float32, kind="ExternalInput")
    W_attn1 = nc.dram_tensor("W_attn1", (F, H), dt.float32, kind="ExternalInput")
    A2blk = nc.dram_tensor("A2blk", (F, H), dt.float32, kind="ExternalInput")
    gidxT = nc.dram_tensor("gidxT", (128, epad // 16), dt.int16, kind="ExternalInput")
    odt = dt.float8e4 if f16 else dt.float32
    if has_bias:
        brow_in = nc.dram_tensor("brow", (1, F2 + H), dt.float32, kind="ExternalInput")

    outT = nc.dram_tensor("outT", (F, npc), dt.float32, kind="ExternalOutput")

    # ---- internal tables ----
    G = nc.dram_tensor("Gtbl", (128, nchT, grow), gdt, kind="Internal")
    Gflat = G.rearrange("p c g -> (p c) g")

    WB = F2 + H  # 144: [W1 | Wqa | W3A2]
    AOP = mybir.AluOpType

    with TileContext(nc) as tc:
        with tc.tile_pool(name="const", bufs=1) as cpool, \
             tc.tile_pool(name="work", bufs=3) as pool, \
             tc.tile_pool(name="io", bufs=4) as iop, \
             tc.tile_pool(name="psBig", bufs=2, space="PSUM") as psBig, \
             tc.tile_pool(name="psSmall", bufs=2, space="PSUM") as psSmall, \
             tc.tile_pool(name="psB", bufs=2, space="PSUM") as psB, \
             tc.tile_pool(name="psC", bufs=2, space="PSUM") as psC:

            # ---------- constants ----------
            iota4 = cpool.tile([128, 512], dt.float32)
            nc.gpsimd.iota(iota4, pattern=[[1, 512]], channel_multiplier=0,
                           allow_small_or_imprecise_dtypes=True)
            iota_col = cpool.tile([128, 1], dt.float32)
            nc.gpsimd.iota(iota_col, pattern=[[1, 1]], channel_multiplier=1,
                           allow_small_or_imprecise_dtypes=True)
            ident = cpool.tile([128, 128], dt.float32)
            nc.vector.tensor_scalar(out=ident, in0=iota4[:, 0:128],
                                    scalar1=iota_col[:, :], scalar2=None,
                                    op0=AOP.is_equal)
            if f16:
                ident16 = cpool.tile([128, 128], dt.float16)
                nc.vector.tensor_copy(out=ident16, in_=ident)
            nshift = cpool.tile([128, 1], dt.float32)
            nc.vector.memset(nshift, -shift)

            a2_s = cpool.tile([F, H], dt.float32)
            nc.sync.dma_start(out=a2_s, in_=A2blk[:, :])
            wat_s = cpool.tile([F, H], dt.float32)
            nc.sync.dma_start(out=wat_s, in_=W_attn1[:, :])

            # Wbig = [W1 | Wqa | W3A2]  (phase-1 rhs)
            wbig_s = cpool.tile([F, WB], dt.float32)
            nc.sync.dma_start(out=wbig_s[:, 0:F], in_=W_path[0:F, :])
            w1t_s = cpool.tile([F, F], dt.float32, tag="wtmp")
            nc.sync.dma_start(out=w1t_s, in_=W_pathT[:, 0:F])
            pw1 = psSmall.tile([F, H], dt.float32, tag="small")
            nc.tensor.matmul(pw1, lhsT=w1t_s, rhs=a2_s, start=True, stop=True)
            nc.vector.tensor_tensor(out=wbig_s[:, F:F2], in0=pw1, in1=wat_s,
                                    op=AOP.add)
            w3t_s = cpool.tile([F, F], dt.float32, tag="wtmp2")
            nc.sync.dma_start(out=w3t_s, in_=W_pathT[:, 2 * F:3 * F])
            pw3 = psSmall.tile([F, H], dt.float32, tag="small")
            nc.tensor.matmul(pw3, lhsT=w3t_s, rhs=a2_s, start=True, stop=True)
            nc.vector.tensor_copy(out=wbig_s[:, F2:WB], in_=pw3)
            wbig_e = cpool.tile([F, WB], edt)
            nc.vector.tensor_copy(out=wbig_e, in_=wbig_s)

            # W2cat = [W2 | W2A2]  (phase-2 rhs, edge dtype)
            w2cat_s = cpool.tile([F, F2], edt)
            w2f_s = cpool.tile([F, F], dt.float32, tag="wtmp3")
            nc.sync.dma_start(out=w2f_s, in_=W_path[F:2 * F, :])
            w2t_s = cpool.tile([F, F], dt.float32, tag="wtmp4")
            nc.sync.dma_start(out=w2t_s, in_=W_pathT[:, F:2 * F])
            pw2 = psSmall.tile([F, H], dt.float32, tag="small")
            nc.tensor.matmul(pw2, lhsT=w2t_s, rhs=a2_s, start=True, stop=True)
            nc.vector.tensor_copy(out=w2cat_s[:, 0:F], in_=w2f_s)
            nc.vector.tensor_copy(out=w2cat_s[:, F:F2], in_=pw2)

            # W3 (phase-3 lhsT, edge dtype)
            w3_s = cpool.tile([F, F], dt.float32)
            nc.sync.dma_start(out=w3_s, in_=W_path[2 * F:3 * F, :])
            w3_e = cpool.tile([F, F], edt)
            nc.vector.tensor_copy(out=w3_e, in_=w3_s)

            if has_bias:
                brow_s = cpool.tile([1, WB], dt.float32)
                nc.sync.dma_start(out=brow_s, in_=brow_in[:, :])
                ones_row = cpool.tile([1, 128], dt.float32)
                nc.vector.memset(ones_row, 1.0)

            gidx_s = cpool.tile([128, epad // 16], dt.int16)
            nc.sync.dma_start(out=gidx_s, in_=gidxT[:, :])
            pid = nc.partition_id()

            # R table lives in SBUF: [128, global chunk, H]; chunk = node//128
            R_all = cpool.tile([128, NCORES * nb, H], edt)
            nc.vector.memset(R_all, 0.0)

            # ---------- phase 1: node tables (3 chunks per PSUM bank) ----------
            for c3 in range((nch + 2) // 3):
                k = min(3, nch - c3 * 3)
                nft_ch = pool.tile([128, 384], edt, tag="nftch", bufs=6)
                nc.sync.dma_start(out=nft_ch[:, 0:k * 128],
                                  in_=nftT_full[:, c3 * 384:c3 * 384 + k * 128])
                p1pool, p1tag = [(psBig, "big"), (psB, "aggB"),
                                 (psC, "outC")][c3 % 3]
                gch = p1pool.tile([128, 3, WB], dt.float32, tag=p1tag)
                for i in range(k):
                    nc.tensor.matmul(gch[:, i, :], lhsT=nft_ch[:, i * 128:(i + 1) * 128],
                                     rhs=wbig_e, start=True, stop=not has_bias)
                    if has_bias:
                        nc.tensor.matmul(gch[:, i, :], lhsT=ones_row, rhs=brow_s,
                                         start=False, stop=True)
                gst = pool.tile([128, 3, grow], gdt, tag="gst", bufs=6)
                if f16:
                    nc.vector.memset(gst[:, :, F2 + H:grow], 0.0)
                    nc.vector.tensor_copy(out=gst[:, 0:k, 0:F],
                                          in_=gch[:, 0:k, 0:F])
                    nc.vector.tensor_copy(
                        out=gst[:, :, :].bitcast(dt.float32)[:, 0:k, 64:72],
                        in_=gch[:, 0:k, F:F2])
                else:
                    nc.vector.memset(gst[:, :, F2:grow], 0.0)
                    nc.vector.tensor_copy(out=gst[:, 0:k, 0:F2],
                                          in_=gch[:, 0:k, 0:F2])
                nc.vector.tensor_copy(out=R_all[:, c3 * 3:c3 * 3 + k, :],
                                      in_=gch[:, 0:k, F2:WB])
                nc.sync.dma_start(out=G[:, c3 * 3:c3 * 3 + k, :],
                                  in_=gst[:, 0:k, :])

            # own blocks' r values: [128, nb, H] via one runtime-offset copy
            R_own = cpool.tile([128, nb, H], edt)
            nc.sync.dma_start(out=R_own,
                              in_=R_all[:, bass.ds(pid * nb, nb), :])

            # sentinel row (flat idx nchT-1): huge negative qa -> u = 0
            sent = cpool.tile([1, grow], gdt)
            nc.vector.memset(sent, 0.0)
            if f16:
                nc.vector.memset(sent[:, :].bitcast(dt.float32)[:, 64:72], -10000.0)
            else:
                nc.vector.memset(sent[:, F:F2], -10000.0)
            nc.sync.dma_start(out=G[0:1, nchT - 1, :], in_=sent)

            # ---------- phase 2 + 3 ----------
            psb_cur = None
            gg = eft_ld = pch = ptch = rblk = None
            psa4 = psl = None
            for t in range(ntiles):
                b, j, last = blk_of[t], jpos[t], jlast[t]
                t4 = t % 4
                tg = t % GCHUNK
                if tg == 0:
                    w = min(GCHUNK * 128, (ntiles - t) * 128)
                    epp_ld = iop.tile([128, eppB], dt.uint8, tag="epp")
                    nc.sync.dma_start(out=epp_ld,
                                      in_=epp[:, t // GCHUNK, :])
                    eft_ld = epp_ld[:, 0:eppE].bitcast(edt)
                    pch = epp_ld[:, eppE:eppE + eppO].bitcast(odt)
                    ptch = epp_ld[:, eppE + eppO:eppB].bitcast(odt)
                    nidx = w
                    gg = iop.tile([128, GCHUNK, grow], gdt, tag="gg")
                    nc.gpsimd.dma_gather(
                        gg[:, 0:nidx // 128, :], Gflat[:, :],
                        gidx_s[:, (t // GCHUNK) * (GCHUNK * 8):
                               (t // GCHUNK) * (GCHUNK * 8) + nidx // 16],
                        num_idxs=nidx, num_idxs_reg=nidx, elem_size=grow,
                        queue_num=(t // GCHUNK) % 4)
                rblk = R_own[:, b, :]
                if t4 == 0:
                    psa4 = psBig.tile([128, 512], dt.float32, tag="big")
                    psl = psSmall.tile([128, 32], dt.float32, tag="small")

                # main matmuls: stationary = transposed edge tile
                et = eft_ld[:, tg * 128:(tg + 1) * 128]
                nc.tensor.matmul(psa4[:, t4 * 128:(t4 + 1) * 128], lhsT=et,
                                 rhs=w2cat_s[:, 0:F], start=True, stop=True,
                                 skip_group_check=True)
                nc.tensor.matmul(psl[:, t4 * 8:(t4 + 1) * 8], lhsT=et,
                                 rhs=w2cat_s[:, F:F2], start=True, stop=False,
                                 skip_group_check=True)
                # += r[dst] via host-built transposed one-hot
                nc.tensor.matmul(psl[:, t4 * 8:(t4 + 1) * 8],
                                 lhsT=ptch[:, tg * 128:(tg + 1) * 128],
                                 rhs=rblk, start=False, stop=True,
                                 skip_group_check=True)

                if t4 != (min(4, ntiles - t + t4) - 1):
                    continue
                # ---- batch epilogue: n4 tiles of logits and messages ----
                n4 = t4 + 1
                w4 = n4 * 8
                tb = t - t4
                h4 = tb % GCHUNK  # gather-chunk offset of this compute batch
                if f16:
                    qa4 = gg[:, :, :].bitcast(dt.float32)[:, h4:h4 + n4, 64:72]
                else:
                    qa4 = gg[:, h4:h4 + n4, F:F2]
                z4 = pool.tile([128, 32], dt.float32, tag="z4")
                nc.vector.tensor_tensor(
                    out=z4[:, 0:w4].rearrange("p (k h) -> p k h", h=H),
                    in0=psl[:, 0:w4].rearrange("p (k h) -> p k h", h=H),
                    in1=qa4, op=AOP.add)
                a4 = pool.tile([128, 32], dt.float32, tag="a4")
                nc.vector.scalar_tensor_tensor(
                    out=a4[:, 0:w4], in0=z4[:, 0:w4], scalar=0.01,
                    in1=z4[:, 0:w4], op0=AOP.mult, op1=AOP.max)
                msgu4 = pool.tile([128, 4, F2], edt, tag="msgu4")
                nc.scalar.activation(
                    msgu4[:, 0:n4, F:F2],
                    a4[:, 0:w4].rearrange("p (k h) -> p k h", h=H),
                    mybir.ActivationFunctionType.Exp,
                    bias=nshift[:, :])
                part4 = pool.tile([128, 512], edt, tag="part4")
                nc.vector.tensor_tensor(
                    out=part4[:, 0:n4 * 128].rearrange("p (k c) -> p k c", k=n4),
                    in0=psa4[:, 0:n4 * 128].rearrange("p (k c) -> p k c", k=n4),
                    in1=gg[:, h4:h4 + n4, 0:F], op=AOP.add)
                nc.vector.tensor_tensor(
                    out=msgu4[:, 0:n4, 0:F].rearrange("p k (h d) -> p k h d", h=H),
                    in0=part4[:, 0:n4 * 128].rearrange("p (k h d) -> p k h d",
                                                       k=n4, h=H),
                    in1=msgu4[:, 0:n4, F:F2][:, :, :, None]
                        .broadcast_to((128, n4, H, DH)),
                    op=AOP.mult)

                # scatter each tile of the batch into its block accumulator
                for k in range(n4):
                    tk = tb + k
                    bb, jj, ll = blk_of[tk], jpos[tk], jlast[tk]
                    tkg = tk % GCHUNK
                    if jj == 0:
                        psb_cur = psB.tile([128, F2], dt.float32, tag="aggB")
                    nc.tensor.matmul(psb_cur,
                                     lhsT=pch[:, tkg * 128:(tkg + 1) * 128],
                                     rhs=msgu4[:, k, :],
                                     start=(jj == 0), stop=ll,
                                     skip_group_check=True)
                    if not ll:
                        continue
                    # ---------- phase 3 for block bb ----------
                    ss = pool.tile([128, H], dt.float32, tag="ss")
                    nc.vector.tensor_scalar(out=ss, in0=psb_cur[:, F:F2],
                                            scalar1=1e-30, scalar2=None,
                                            op0=AOP.max)
                    inv = pool.tile([128, H], dt.float32, tag="inv")
                    nc.vector.reciprocal(inv, ss)
                    mn = pool.tile([128, F], dt.float32, tag="mn")
                    nc.vector.tensor_tensor(
                        out=mn[:, :].rearrange("p (h d) -> p h d", h=H),
                        in0=psb_cur[:, 0:F].rearrange("p (h d) -> p h d", h=H),
                        in1=inv[:, :, None].broadcast_to((128, H, DH)),
                        op=AOP.mult)
                    ncc = pool.tile([128, 2, 128], edt, tag="ncc")
                    nc.sync.dma_start(out=ncc, in_=nftT_cc[:, bb, :, :])
                    nfsm = ncc[:, 0, :]
                    nfs = ncc[:, 1, :]
                    psc = psC.tile([128, 128], dt.float32, tag="outC")
                    nc.tensor.matmul(psc, lhsT=w3_e, rhs=nfsm,
                                     start=True, stop=False)
                    nc.tensor.matmul(psc, lhsT=mn, rhs=ident, is_transpose=True,
                                     start=False, stop=True)
                    oc = pool.tile([128, 128], dt.float32, tag="oc")
                    nc.vector.tensor_tensor(out=oc, in0=psc, in1=nfs, op=AOP.add)
                    oc2 = pool.tile([128, 128], dt.float32, tag="oc2")
                    nc.scalar.activation(oc2, oc,
                                         mybir.ActivationFunctionType.Relu)
                    nc.sync.dma_start(out=outT[:, bb * 128:(bb + 1) * 128],
                                      in_=oc2)

    nc.compile()
    return nc


def prep_inputs(nft, eft, W_path, b_path, W_attn1, attn2, src, dst,
                npc, tiles, prec=PREC):
    """Host-side sharding/relayout. Returns (in_maps, meta, has_bias)."""
    n_nodes = nft.shape[0]
    nb = npc // 128
    ntiles = sum(tiles)
    toff = np.concatenate([[0], np.cumsum(tiles)])  # tile offset per block
    epad = ntiles * 128
    ngrp = (ntiles + GCHUNK - 1) // GCHUNK
    nch = (n_nodes + 127) // 128
    npad = nch * 128 + 128
    edt_np = np.float16 if prec == "f16" else np.float32
    if prec == "f16":
        import ml_dtypes
        odt_np = ml_dtypes.float8_e4m3fn
    else:
        odt_np = np.float32

    nft = np.ascontiguousarray(nft, dtype=np.float32)
    eft = np.ascontiguousarray(eft, dtype=np.float32)
    perm = np.argsort(dst, kind="stable")
    sdst = dst[perm].astype(np.int64)
    ssrc = src[perm].astype(np.int64)
    seft = eft[perm]

    nftT_full = np.zeros((F, npad), dtype=edt_np)
    nftT_full[:, :n_nodes] = nft.T

    a2 = np.asarray(attn2, dtype=np.float32).reshape(H, DH)
    A2blk = np.zeros((F, H), dtype=np.float32)
    for h in range(H):
        A2blk[h * DH:(h + 1) * DH, h] = a2[h]

    has_bias = bool(np.any(np.asarray(b_path) != 0))
    brow = None
    if has_bias:
        b = np.asarray(b_path, dtype=np.float32).reshape(F)
        brow = np.zeros((1, F2 + H), dtype=np.float32)
        brow[0, 0:F] = b
        brow[0, F:F2] = b @ A2blk
    Wp = np.ascontiguousarray(W_path, dtype=np.float32)

    in_maps = []
    meta = []
    for c in range(NCORES):
        lo = c * npc
        hi = min((c + 1) * npc, n_nodes)

        eftT_c = np.zeros((F, epad), dtype=edt_np)
        nchT = nch + 1
        gidx = np.full(epad, nchT - 1, dtype=np.int64)  # sentinel flat idx
        dstloc = np.zeros(epad, dtype=np.int64)

        for b_i in range(nb):
            base = lo + b_i * 128
            if base >= n_nodes:
                continue
            s = np.searchsorted(sdst, base)
            e = np.searchsorted(sdst, min(base + 128, n_nodes))
            cnt = e - s
            assert cnt <= tiles[b_i] * 128, f"block overflow: {cnt}"
            o = toff[b_i] * 128
            eftT_c[:, o:o + cnt] = seft[s:e].T.astype(edt_np)
            sr = ssrc[s:e]
            gidx[o:o + cnt] = (sr % 128) * nchT + sr // 128  # partition-major
            dstloc[o:o + cnt] = sdst[s:e] - base

        def wrap16(idx):
            w = idx.reshape(-1, 16).T.copy()
            return np.tile(w, (8, 1)).astype(np.int16)

        ee = np.arange(epad)
        Pcat = np.zeros((128, epad), dtype=odt_np)
        Pcat[ee % 128, (ee // 128) * 128 + dstloc] = 1.0
        PTcat = np.zeros((128, epad), dtype=odt_np)
        PTcat[dstloc, ee] = 1.0
        # interleave [eft | Pcat | PTcat] per gather group as raw bytes
        gpad = ngrp * GCHUNK * 128

        def padg(a, dtp):
            out = np.zeros((128, gpad), dtype=dtp)
            out[:, :epad] = a
            return np.ascontiguousarray(
                out.reshape(128, ngrp, GCHUNK * 128)).view(np.uint8)

        epp = np.concatenate([padg(eftT_c, edt_np), padg(Pcat, odt_np),
                              padg(PTcat, odt_np)], axis=2)

        nftT_c = np.zeros((F, npc), dtype=edt_np)
        nftT_c[:, :hi - lo] = nft[lo:hi].T
        e_lo = np.searchsorted(sdst, lo)
        e_hi = np.searchsorted(sdst, hi)
        deg = np.bincount(sdst[e_lo:e_hi] - lo, minlength=npc)[:hi - lo]
        nftT_cm = np.zeros((F, npc), dtype=edt_np)
        nftT_cm[:, :hi - lo] = (nft[lo:hi] * (deg > 0)[:, None]).T
        nftT_cc = np.empty((128, nb, 2, 128), dtype=edt_np)
        nftT_cc[:, :, 0, :] = nftT_cm.reshape(128, nb, 128).transpose(0, 1, 2)
        nftT_cc[:, :, 1, :] = nftT_c.reshape(128, nb, 128).transpose(0, 1, 2)

        m = {
            "nftT_full": nftT_full,
            "nftT_cc": nftT_cc,
            "epp": epp,
            "W_path": Wp,
            "W_pathT": np.ascontiguousarray(Wp.T),
            "W_attn1": np.ascontiguousarray(W_attn1, dtype=np.float32),
            "A2blk": A2blk,
            "gidxT": wrap16(gidx),
        }
        if has_bias:
            m["brow"] = brow
        in_maps.append(m)
        meta.append((lo, hi))
    return in_maps, meta, has_bias


_NC_CACHE = {}


def _get_nc(key, *args, **kw):
    if key not in _NC_CACHE:
        _NC_CACHE[key] = build_nc(*args, **kw)
    return _NC_CACHE[key]


def run(nft, eft, W_path, b_path, W_attn1, attn2, src, dst, trace=False,
        tmpdir=None, prec=PREC):
    n_nodes = nft.shape[0]
    npc = ((n_nodes + NCORES - 1) // NCORES + 127) // 128 * 128
    nb = npc // 128
    dst64 = np.asarray(dst, dtype=np.int64)
    # per-block-slot tile counts: max over cores so the SPMD schedule matches
    cnt = np.bincount(dst64, minlength=((n_nodes + 127) // 128) * 128)
    blocks = cnt.reshape(-1, 128).sum(axis=1)  # edges per global 128-block
    percore = np.zeros((NCORES, nb), dtype=np.int64)
    for c in range(NCORES):
        for b in range(nb):
            g = c * nb + b
            if g < len(blocks):
                percore[c, b] = blocks[g]
    tiles = tuple(int(x) for x in
                  np.maximum(1, -(-percore.max(axis=0) // 128)))

    in_maps, meta, has_bias = prep_inputs(
        np.asarray(nft), np.asarray(eft), np.asarray(W_path),
        np.asarray(b_path), np.asarray(W_attn1), np.asarray(attn2),
        np.asarray(src), dst64, npc, tiles, prec=prec)

    nc = _get_nc((n_nodes, npc, tiles, has_bias, prec),
                 n_nodes, npc, tiles, has_bias, prec=prec)
    kw = {}
    if trace:
        kw = dict(trace=True, tmpdir=tmpdir)
    res = bass_utils.run_bass_kernel_spmd(nc, in_maps,
                                          core_ids=list(range(NCORES)), **kw)

    out = np.empty((n_nodes, F), dtype=np.float32)
    for c, (lo, hi) in enumerate(meta):
        out[lo:hi] = res.results[c]["outT"][:, :hi - lo].T
    return out, res


def kernel(**inputs):
    out, _ = run(**inputs)
    return out


# revision 36
# speedup vs baseline: 1.1547x; 1.1547x over previous
"""GAT message-passing kernel for 8 Trainium2 NeuronCores (Bass/Tile).

Strategy ("route edges by dst ownership"):
  - Host sorts edges by dst, partitions nodes into 8 equal ranges; each core
    owns all edges whose dst falls in its range, so segment-softmax and
    scatter-sum are fully core-local (no collectives).
  - Reassociation: epaths = y1[src] + eft@W2 + y3[dst] + b with y1 = nft@W1.
    Since sum(att)=1 per (node, head), the y3[dst] part of the aggregated
    message is exactly +y3[dst], added once per node in phase 3.  Softmax is
    computed without max-subtraction; a fixed shift exp(a-7) keeps the
    unnormalized weights in fp16 range (softmax is shift-invariant).
  - Phase 1 (per core, all nodes, fp16): G table rows [y1 | qa], R table rows
    [r]; qa = a1 + y1.A2, r = y3.A2.
  - Phase 2, per tile of 128 edges (grouped per owning 128-node dst block,
    per-block tile counts fixed across cores at the max over cores):
    dma_gather G rows by src (2048-idx calls, 4 SWDGE queues); matmuls with
    the transposed edge-feature tile stationary produce eft@W2 (PSUM, 4-tile
    batches) and eft@(W2.A2) logits; r[dst] is added by a one-hot matmul
    whose one-hot is built ON DEVICE (broadcast matmul of dstloc + is_equal
    against a partition iota); logits -> leaky_relu (DVE) -> exp(a-7)
    (ACT, fp16); the scatter one-hot P is also built on device (is_equal of
    a free-dim iota against per-partition dstloc); a scatter matmul
    accumulates [agg | s] for the owning 128-node block.
  - Phase 3 (per node block, fp16 operands): agg/s, transpose via PE,
    += nft@W3 (matmul accumulate) and += nft, relu, store transposed.
"""

import sys
import numpy as np

for _p in ("/opt/trn_rl_repo",):
    if _p not in sys.path:
        sys.path.append(_p)

import concourse.bacc as bacc
import concourse.bass as bass
import concourse.mybir as mybir
from concourse.tile import TileContext
from concourse import bass_utils

F = 128
H = 8
DH = 16
F2 = F + H  # 136
NCORES = 8
EXP_SHIFT = 7.0  # exp(a - shift); softmax-invariant, keeps u in fp16 normal range
PREC = "f16"     # "f16" (fast path) or "f32"
GCHUNK = 8       # tiles per gather call / eft chunk


def build_nc(n_nodes, npc, tiles, has_bias, prec=PREC, debug=False):
    nb = npc // 128                  # node block slots per core
    assert len(tiles) == nb
    ntiles = sum(tiles)              # edge tiles per core
    epad = ntiles * 128              # padded edge count per core
    nch = (n_nodes + 127) // 128     # phase-1 node chunks
    npad = nch * 128 + 128           # +128 rows; sentinel lives at row nch*128
    dt = mybir.dt
    f16 = prec == "f16"
    edt = dt.float16 if f16 else dt.float32
    # G row layout: f16 -> fp16[y1(128) | qa-as-f32-bits(16) | pad] row 512 B
    #               f32 -> fp32[y1(128) | qa(8) | pad] row 768 B
    gdt = dt.float16 if f16 else dt.float32
    grow = 256 if f16 else 192
    nchT = nch + 1                   # chunk slots per partition (+1 sentinel)
    shift = EXP_SHIFT if f16 else 0.0

    # flat tile schedule
    blk_of, jpos, jlast = [], [], []
    for b, tb in enumerate(tiles):
        for j in range(tb):
            blk_of.append(b)
            jpos.append(j)
            jlast.append(j == tb - 1)

    nc = bacc.Bacc("TRN2", target_bir_lowering=False, debug=False,
                   num_devices=NCORES, num_swdge_queues=4)

    # ---- inputs (identical shapes on every core) ----
    nftT_full = nc.dram_tensor("nftT_full", (F, npad), edt, kind="ExternalInput")
    nftT_cc = nc.dram_tensor("nftT_cc", (128, nb, 2, 128), edt, kind="ExternalInput")
    # epp: per gather-group interleave [eft | Pcat | PTcat] as raw bytes
    esz = 2 if f16 else 4
    osz = 1 if f16 else 4
    eppE = GCHUNK * 128 * esz        # eft bytes per partition per group
    eppO = GCHUNK * 128 * osz        # one-hot bytes per partition per group
    eppB = eppE + 2 * eppO
    ngrp = (ntiles + GCHUNK - 1) // GCHUNK
    epp = nc.dram_tensor("epp", (128, ngrp, eppB), dt.uint8, kind="ExternalInput")
    W_path = nc.dram_tensor("W_path", (3 * F, F), dt.float32, kind="ExternalInput")
    W_pathT = nc.dram_tensor("W_pathT", (F, 3 * F), dt.float32, kind="ExternalInput")
    W_attn1 = nc.dram_tensor("W_attn1", (F, H), dt.float32, kind="ExternalInput")
    A2blk = nc.dram_tensor("A2blk", (F, H), dt.float32, kind="ExternalInput")
    gidxT = nc.dram_tensor("gidxT", (128, epad // 16), dt.int16, kind="ExternalInput")
    odt = dt.float8e4 if f16 else dt.float32
    if has_bias:
        brow_in = nc.dram_tensor("brow", (1, F2 + H), dt.float32, kind="ExternalInput")

    outT = nc.dram_tensor("outT", (F, npc), dt.float32, kind="ExternalOutput")

    # ---- internal tables ----
    G = nc.dram_tensor("Gtbl", (128, nchT, grow), gdt, kind="Internal")
    Gflat = G.rearrange("p c g -> (p c) g")

    WB = F2 + H  # 144: [W1 | Wqa | W3A2]
    AOP = mybir.AluOpType

    with TileContext(nc) as tc:
        with tc.tile_pool(name="const", bufs=1) as cpool, \
             tc.tile_pool(name="work", bufs=3) as pool, \
             tc.tile_pool(name="io", bufs=4) as iop, \
             tc.tile_pool(name="psBig", bufs=2, space="PSUM") as psBig, \
             tc.tile_pool(name="psSmall", bufs=2, space="PSUM") as psSmall, \
             tc.tile_pool(name="psB", bufs=2, space="PSUM") as psB, \
             tc.tile_pool(name="psC", bufs=2, space="PSUM") as psC:

            # ---------- constants ----------
            iota4 = cpool.tile([128, 512], dt.float32)
            nc.gpsimd.iota(iota4, pattern=[[1, 512]], channel_multiplier=0,
                           allow_small_or_imprecise_dtypes=True)
            iota_col = cpool.tile([128, 1], dt.float32)
            nc.gpsimd.iota(iota_col, pattern=[[1, 1]], channel_multiplier=1,
                           allow_small_or_imprecise_dtypes=True)
            ident = cpool.tile([128, 128], dt.float32)
            nc.vector.tensor_scalar(out=ident, in0=iota4[:, 0:128],
                                    scalar1=iota_col[:, :], scalar2=None,
                                    op0=AOP.is_equal)
            if f16:
                ident16 = cpool.tile([128, 128], dt.float16)
                nc.vector.tensor_copy(out=ident16, in_=ident)
            nshift = cpool.tile([128, 1], dt.float32)
            nc.vector.memset(nshift, -shift)

            a2_s = cpool.tile([F, H], dt.float32)
            nc.sync.dma_start(out=a2_s, in_=A2blk[:, :])
            wat_s = cpool.tile([F, H], dt.float32)
            nc.sync.dma_start(out=wat_s, in_=W_attn1[:, :])

            # Wbig = [W1 | Wqa | W3A2]  (phase-1 rhs)
            wbig_s = cpool.tile([F, WB], dt.float32)
            nc.sync.dma_start(out=wbig_s[:, 0:F], in_=W_path[0:F, :])
            w1t_s = cpool.tile([F, F], dt.float32, tag="wtmp")
            nc.sync.dma_start(out=w1t_s, in_=W_pathT[:, 0:F])
            pw1 = psSmall.tile([F, H], dt.float32, tag="small")
            nc.tensor.matmul(pw1, lhsT=w1t_s, rhs=a2_s, start=True, stop=True)
            nc.vector.tensor_tensor(out=wbig_s[:, F:F2], in0=pw1, in1=wat_s,
                                    op=AOP.add)
            w3t_s = cpool.tile([F, F], dt.float32, tag="wtmp2")
            nc.sync.dma_start(out=w3t_s, in_=W_pathT[:, 2 * F:3 * F])
            pw3 = psSmall.tile([F, H], dt.float32, tag="small")
            nc.tensor.matmul(pw3, lhsT=w3t_s, rhs=a2_s, start=True, stop=True)
            nc.vector.tensor_copy(out=wbig_s[:, F2:WB], in_=pw3)
            wbig_e = cpool.tile([F, WB], edt)
            nc.vector.tensor_copy(out=wbig_e, in_=wbig_s)

            # W2cat = [W2 | W2A2]  (phase-2 rhs, edge dtype)
            w2cat_s = cpool.tile([F, F2], edt)
            w2f_s = cpool.tile([F, F], dt.float32, tag="wtmp3")
            nc.sync.dma_start(out=w2f_s, in_=W_path[F:2 * F, :])
            w2t_s = cpool.tile([F, F], dt.float32, tag="wtmp4")
            nc.sync.dma_start(out=w2t_s, in_=W_pathT[:, F:2 * F])
            pw2 = psSmall.tile([F, H], dt.float32, tag="small")
            nc.tensor.matmul(pw2, lhsT=w2t_s, rhs=a2_s, start=True, stop=True)
            nc.vector.tensor_copy(out=w2cat_s[:, 0:F], in_=w2f_s)
            nc.vector.tensor_copy(out=w2cat_s[:, F:F2], in_=pw2)

            # W3 (phase-3 lhsT, edge dtype)
            w3_s = cpool.tile([F, F], dt.float32)
            nc.sync.dma_start(out=w3_s, in_=W_path[2 * F:3 * F, :])
            w3_e = cpool.tile([F, F], edt)
            nc.vector.tensor_copy(out=w3_e, in_=w3_s)

            if has_bias:
                brow_s = cpool.tile([1, WB], dt.float32)
                nc.sync.dma_start(out=brow_s, in_=brow_in[:, :])
                ones_row = cpool.tile([1, 128], dt.float32)
                nc.vector.memset(ones_row, 1.0)

            gidx_s = cpool.tile([128, epad // 16], dt.int16)
            nc.sync.dma_start(out=gidx_s, in_=gidxT[:, :])
            pid = nc.partition_id()

            # R table lives in SBUF: [128, global chunk, H]; chunk = node//128
            R_all = cpool.tile([128, NCORES * nb, H], edt)
            nc.vector.memset(R_all, 0.0)

            # ---------- phase 1: node tables (3 chunks per PSUM bank) ----------
            for c3 in range((nch + 2) // 3):
                k = min(3, nch - c3 * 3)
                nft_ch = pool.tile([128, 384], edt, tag="nftch", bufs=6)
                nc.sync.dma_start(out=nft_ch[:, 0:k * 128],
                                  in_=nftT_full[:, c3 * 384:c3 * 384 + k * 128])
                p1pool, p1tag = [(psBig, "big"), (psB, "aggB"),
                                 (psC, "outC")][c3 % 3]
                gch = p1pool.tile([128, 3, WB], dt.float32, tag=p1tag)
                for i in range(k):
                    nc.tensor.matmul(gch[:, i, :], lhsT=nft_ch[:, i * 128:(i + 1) * 128],
                                     rhs=wbig_e, start=True, stop=not has_bias)
                    if has_bias:
                        nc.tensor.matmul(gch[:, i, :], lhsT=ones_row, rhs=brow_s,
                                         start=False, stop=True)
                gst = pool.tile([128, 3, F2], gdt, tag="gst", bufs=6)
                nc.scalar.activation(gst[:, 0:k, 0:F2], gch[:, 0:k, 0:F2],
                                     mybir.ActivationFunctionType.Copy)
                nc.vector.tensor_copy(out=R_all[:, c3 * 3:c3 * 3 + k, :],
                                      in_=gch[:, 0:k, F2:WB])
                nc.sync.dma_start(out=G[:, c3 * 3:c3 * 3 + k, 0:F2],
                                  in_=gst[:, 0:k, :])

            # own blocks' r values: [128, nb, H] via one runtime-offset copy
            R_own = cpool.tile([128, nb, H], edt)
            nc.sync.dma_start(out=R_own,
                              in_=R_all[:, bass.ds(pid * nb, nb), :])

            # sentinel row (flat idx nchT-1): huge negative qa -> u = 0
            sent = cpool.tile([1, grow], gdt)
            nc.vector.memset(sent, 0.0)
            nc.vector.memset(sent[:, F:F2], -10000.0)
            nc.sync.dma_start(out=G[0:1, nchT - 1, :], in_=sent)

            # ---------- phase 2 + 3 ----------
            psb_cur = None
            gg = eft_ld = pch = ptch = rblk = None
            psa4 = psl = None
            for t in range(ntiles):
                b, j, last = blk_of[t], jpos[t], jlast[t]
                t4 = t % 4
                tg = t % GCHUNK
                if tg == 0:
                    w = min(GCHUNK * 128, (ntiles - t) * 128)
                    epp_ld = iop.tile([128, eppB], dt.uint8, tag="epp")
                    nc.sync.dma_start(out=epp_ld,
                                      in_=epp[:, t // GCHUNK, :])
                    eft_ld = epp_ld[:, 0:eppE].bitcast(edt)
                    pch = epp_ld[:, eppE:eppE + eppO].bitcast(odt)
                    ptch = epp_ld[:, eppE + eppO:eppB].bitcast(odt)
                    nidx = w
                    gg = iop.tile([128, GCHUNK, grow], gdt, tag="gg")
                    nc.gpsimd.dma_gather(
                        gg[:, 0:nidx // 128, :], Gflat[:, :],
                        gidx_s[:, (t // GCHUNK) * (GCHUNK * 8):
                               (t // GCHUNK) * (GCHUNK * 8) + nidx // 16],
                        num_idxs=nidx, num_idxs_reg=nidx, elem_size=grow,
                        queue_num=(t // GCHUNK) % 4)
                rblk = R_own[:, b, :]
                if t4 == 0:
                    psa4 = psBig.tile([128, 512], dt.float32, tag="big")
                    psl = psSmall.tile([128, 32], dt.float32, tag="small")

                # main matmuls: stationary = transposed edge tile
                et = eft_ld[:, tg * 128:(tg + 1) * 128]
                nc.tensor.matmul(psa4[:, t4 * 128:(t4 + 1) * 128], lhsT=et,
                                 rhs=w2cat_s[:, 0:F], start=True, stop=True,
                                 skip_group_check=True)
                nc.tensor.matmul(psl[:, t4 * 8:(t4 + 1) * 8], lhsT=et,
                                 rhs=w2cat_s[:, F:F2], start=True, stop=False,
                                 skip_group_check=True)
                # += r[dst] via host-built transposed one-hot
                nc.tensor.matmul(psl[:, t4 * 8:(t4 + 1) * 8],
                                 lhsT=ptch[:, tg * 128:(tg + 1) * 128],
                                 rhs=rblk, start=False, stop=True,
                                 skip_group_check=True)

                if t4 != (min(4, ntiles - t + t4) - 1):
                    continue
                # ---- batch epilogue: n4 tiles of logits and messages ----
                n4 = t4 + 1
                w4 = n4 * 8
                tb = t - t4
                h4 = tb % GCHUNK  # gather-chunk offset of this compute batch
                qa4 = gg[:, h4:h4 + n4, F:F2]
                z4 = pool.tile([128, 32], dt.float32, tag="z4")
                nc.vector.tensor_tensor(
                    out=z4[:, 0:w4].rearrange("p (k h) -> p k h", h=H),
                    in0=psl[:, 0:w4].rearrange("p (k h) -> p k h", h=H),
                    in1=qa4, op=AOP.add)
                a4 = pool.tile([128, 32], dt.float32, tag="a4")
                nc.vector.scalar_tensor_tensor(
                    out=a4[:, 0:w4], in0=z4[:, 0:w4], scalar=0.01,
                    in1=z4[:, 0:w4], op0=AOP.mult, op1=AOP.max)
                msgu4 = pool.tile([128, 4, F2], edt, tag="msgu4")
                nc.scalar.activation(
                    msgu4[:, 0:n4, F:F2],
                    a4[:, 0:w4].rearrange("p (k h) -> p k h", h=H),
                    mybir.ActivationFunctionType.Exp,
                    bias=nshift[:, :])
                part4 = pool.tile([128, 512], edt, tag="part4")
                nc.vector.tensor_tensor(
                    out=part4[:, 0:n4 * 128].rearrange("p (k c) -> p k c", k=n4),
                    in0=psa4[:, 0:n4 * 128].rearrange("p (k c) -> p k c", k=n4),
                    in1=gg[:, h4:h4 + n4, 0:F], op=AOP.add)
                nc.vector.tensor_tensor(
                    out=msgu4[:, 0:n4, 0:F].rearrange("p k (h d) -> p k h d", h=H),
                    in0=part4[:, 0:n4 * 128].rearrange("p (k h d) -> p k h d",
                                                       k=n4, h=H),
                    in1=msgu4[:, 0:n4, F:F2][:, :, :, None]
                        .broadcast_to((128, n4, H, DH)),
                    op=AOP.mult)

                # scatter each tile of the batch into its block accumulator
                for k in range(n4):
                    tk = tb + k
                    bb, jj, ll = blk_of[tk], jpos[tk], jlast[tk]
                    tkg = tk % GCHUNK
                    if jj == 0:
                        psb_cur = psB.tile([128, F2], dt.float32, tag="aggB")
                    nc.tensor.matmul(psb_cur,
                                     lhsT=pch[:, tkg * 128:(tkg + 1) * 128],
                                     rhs=msgu4[:, k, :],
                                     start=(jj == 0), stop=ll,
                                     skip_group_check=True)
                    if not ll:
                        continue
                    # ---------- phase 3 for block bb ----------
                    ss = pool.tile([128, H], dt.float32, tag="ss")
                    nc.vector.tensor_scalar(out=ss, in0=psb_cur[:, F:F2],
                                            scalar1=1e-30, scalar2=None,
                                            op0=AOP.max)
                    inv = pool.tile([128, H], dt.float32, tag="inv")
                    nc.vector.reciprocal(inv, ss)
                    mn = pool.tile([128, F], dt.float32, tag="mn")
                    nc.vector.tensor_tensor(
                        out=mn[:, :].rearrange("p (h d) -> p h d", h=H),
                        in0=psb_cur[:, 0:F].rearrange("p (h d) -> p h d", h=H),
                        in1=inv[:, :, None].broadcast_to((128, H, DH)),
                        op=AOP.mult)
                    ncc = pool.tile([128, 2, 128], edt, tag="ncc")
                    nc.sync.dma_start(out=ncc, in_=nftT_cc[:, bb, :, :])
                    nfsm = ncc[:, 0, :]
                    nfs = ncc[:, 1, :]
                    psc = psC.tile([128, 128], dt.float32, tag="outC")
                    nc.tensor.matmul(psc, lhsT=w3_e, rhs=nfsm,
                                     start=True, stop=False)
                    nc.tensor.matmul(psc, lhsT=mn, rhs=ident, is_transpose=True,
                                     start=False, stop=True)
                    oc = pool.tile([128, 128], dt.float32, tag="oc")
                    nc.vector.tensor_tensor(out=oc, in0=psc, in1=nfs, op=AOP.add)
                    oc2 = pool.tile([128, 128], dt.float32, tag="oc2")
                    nc.scalar.activation(oc2, oc,
                                         mybir.ActivationFunctionType.Relu)
                    nc.sync.dma_start(out=outT[:, bb * 128:(bb + 1) * 128],
                                      in_=oc2)

    nc.compile()
    return nc


def prep_inputs(nft, eft, W_path, b_path, W_attn1, attn2, src, dst,
                npc, tiles, prec=PREC):
    """Host-side sharding/relayout. Returns (in_maps, meta, has_bias)."""
    n_nodes = nft.shape[0]
    nb = npc // 128
    ntiles = sum(tiles)
    toff = np.concatenate([[0], np.cumsum(tiles)])  # tile offset per block
    epad = ntiles * 128
    ngrp = (ntiles + GCHUNK - 1) // GCHUNK
    nch = (n_nodes + 127) // 128
    npad = nch * 128 + 128
    edt_np = np.float16 if prec == "f16" else np.float32
    if prec == "f16":
        import ml_dtypes
        odt_np = ml_dtypes.float8_e4m3fn
    else:
        odt_np = np.float32

    nft = np.ascontiguousarray(nft, dtype=np.float32)
    eft = np.ascontiguousarray(eft, dtype=np.float32)
    perm = np.argsort(dst, kind="stable")
    sdst = dst[perm].astype(np.int64)
    ssrc = src[perm].astype(np.int64)
    seft = eft[perm]

    nftT_full = np.zeros((F, npad), dtype=edt_np)
    nftT_full[:, :n_nodes] = nft.T

    a2 = np.asarray(attn2, dtype=np.float32).reshape(H, DH)
    A2blk = np.zeros((F, H), dtype=np.float32)
    for h in range(H):
        A2blk[h * DH:(h + 1) * DH, h] = a2[h]

    has_bias = bool(np.any(np.asarray(b_path) != 0))
    brow = None
    if has_bias:
        b = np.asarray(b_path, dtype=np.float32).reshape(F)
        brow = np.zeros((1, F2 + H), dtype=np.float32)
        brow[0, 0:F] = b
        brow[0, F:F2] = b @ A2blk
    Wp = np.ascontiguousarray(W_path, dtype=np.float32)

    in_maps = []
    meta = []
    for c in range(NCORES):
        lo = c * npc
        hi = min((c + 1) * npc, n_nodes)

        eftT_c = np.zeros((F, epad), dtype=edt_np)
        nchT = nch + 1
        gidx = np.full(epad, nchT - 1, dtype=np.int64)  # sentinel flat idx
        dstloc = np.zeros(epad, dtype=np.int64)

        for b_i in range(nb):
            base = lo + b_i * 128
            if base >= n_nodes:
                continue
            s = np.searchsorted(sdst, base)
            e = np.searchsorted(sdst, min(base + 128, n_nodes))
            cnt = e - s
            assert cnt <= tiles[b_i] * 128, f"block overflow: {cnt}"
            o = toff[b_i] * 128
            eftT_c[:, o:o + cnt] = seft[s:e].T.astype(edt_np)
            sr = ssrc[s:e]
            gidx[o:o + cnt] = (sr % 128) * nchT + sr // 128  # partition-major
            dstloc[o:o + cnt] = sdst[s:e] - base

        def wrap16(idx):
            w = idx.reshape(-1, 16).T.copy()
            return np.tile(w, (8, 1)).astype(np.int16)

        ee = np.arange(epad)
        Pcat = np.zeros((128, epad), dtype=odt_np)
        Pcat[ee % 128, (ee // 128) * 128 + dstloc] = 1.0
        PTcat = np.zeros((128, epad), dtype=odt_np)
        PTcat[dstloc, ee] = 1.0
        # interleave [eft | Pcat | PTcat] per gather group as raw bytes
        gpad = ngrp * GCHUNK * 128

        def padg(a, dtp):
            out = np.zeros((128, gpad), dtype=dtp)
            out[:, :epad] = a
            return np.ascontiguousarray(
                out.reshape(128, ngrp, GCHUNK * 128)).view(np.uint8)

        epp = np.concatenate([padg(eftT_c, edt_np), padg(Pcat, odt_np),
                              padg(PTcat, odt_np)], axis=2)

        nftT_c = np.zeros((F, npc), dtype=edt_np)
        nftT_c[:, :hi - lo] = nft[lo:hi].T
        e_lo = np.searchsorted(sdst, lo)
        e_hi = np.searchsorted(sdst, hi)
        deg = np.bincount(sdst[e_lo:e_hi] - lo, minlength=npc)[:hi - lo]
        nftT_cm = np.zeros((F, npc), dtype=edt_np)
        nftT_cm[:, :hi - lo] = (nft[lo:hi] * (deg > 0)[:, None]).T
        nftT_cc = np.empty((128, nb, 2, 128), dtype=edt_np)
        nftT_cc[:, :, 0, :] = nftT_cm.reshape(128, nb, 128).transpose(0, 1, 2)
        nftT_cc[:, :, 1, :] = nftT_c.reshape(128, nb, 128).transpose(0, 1, 2)

        m = {
            "nftT_full": nftT_full,
            "nftT_cc": nftT_cc,
            "epp": epp,
            "W_path": Wp,
            "W_pathT": np.ascontiguousarray(Wp.T),
            "W_attn1": np.ascontiguousarray(W_attn1, dtype=np.float32),
            "A2blk": A2blk,
            "gidxT": wrap16(gidx),
        }
        if has_bias:
            m["brow"] = brow
        in_maps.append(m)
        meta.append((lo, hi))
    return in_maps, meta, has_bias


_NC_CACHE = {}


def _get_nc(key, *args, **kw):
    if key not in _NC_CACHE:
        _NC_CACHE[key] = build_nc(*args, **kw)
    return _NC_CACHE[key]


def run(nft, eft, W_path, b_path, W_attn1, attn2, src, dst, trace=False,
        tmpdir=None, prec=PREC):
    n_nodes = nft.shape[0]
    npc = ((n_nodes + NCORES - 1) // NCORES + 127) // 128 * 128
    nb = npc // 128
    dst64 = np.asarray(dst, dtype=np.int64)
    # per-block-slot tile counts: max over cores so the SPMD schedule matches
    cnt = np.bincount(dst64, minlength=((n_nodes + 127) // 128) * 128)
    blocks = cnt.reshape(-1, 128).sum(axis=1)  # edges per global 128-block
    percore = np.zeros((NCORES, nb), dtype=np.int64)
    for c in range(NCORES):
        for b in range(nb):
            g = c * nb + b
            if g < len(blocks):
                percore[c, b] = blocks[g]
    tiles = tuple(int(x) for x in
                  np.maximum(1, -(-percore.max(axis=0) // 128)))

    in_maps, meta, has_bias = prep_inputs(
        np.asarray(nft), np.asarray(eft), np.asarray(W_path),
        np.asarray(b_path), np.asarray(W_attn1), np.asarray(attn2),
        np.asarray(src), dst64, npc, tiles, prec=prec)

    nc = _get_nc((n_nodes, npc, tiles, has_bias, prec),
                 n_nodes, npc, tiles, has_bias, prec=prec)
    kw = {}
    if trace:
        kw = dict(trace=True, tmpdir=tmpdir)
    res = bass_utils.run_bass_kernel_spmd(nc, in_maps,
                                          core_ids=list(range(NCORES)), **kw)

    out = np.empty((n_nodes, F), dtype=np.float32)
    for c, (lo, hi) in enumerate(meta):
        out[lo:hi] = res.results[c]["outT"][:, :hi - lo].T
    return out, res


def kernel(**inputs):
    out, _ = run(**inputs)
    return out
